# revision 1
# baseline (speedup 1.0000x reference)
"""Trainium2 Bass kernel for nn_MultiHeadAttention (B=2, N=M=2048, D=1024, H=16).

Sharding: 8 cores = 2 batches x 4 head-groups (4 heads per core, tensor-parallel
over the head dim of Wq/Wk/Wv/Wp).  Each core computes a partial output
projection [N, D]; the host sums the 4 partials per batch and adds bp.

Per-core dataflow (all layouts chosen so the PE contracts over partitions;
fp16 operands for every matmul -- fp32 streams 4x slower through the PE):
  - host passes X_q^T, X_kv^T ([D, N] f16), mask^T ([M, N] f16 {0,1}) and
    per-core f16 weight slices.
  - K^T[dh, m], Q^T[dh, n] via matmul(lhsT=W chunk, rhs=X^T chunk), f32 PSUM,
    bias added in f32 by ACT during evacuation, stored f16.
  - V[m, dh] via matmul(lhsT=X^T chunk, rhs=Wv), bias via an extra ones-outer
    matmul accumulated into PSUM; stored f16 with a ones column per head.
  - S^T tile [m=128, n] = matmul(lhsT=K^T slice, rhs=Q^T); exp on ACT with
    the 1/sqrt(dh) scale folded in; mask multiply on DVE (f16, 2x mode).
  - O'^T[dh+1, n] accumulated over m-chunks: matmul(lhsT=V~[m,65], rhs=E^T);
    row 64 = softmax denominator (ones column trick).  Loop order (nh, g, m)
    so each mask half is DMA'd once.
  - normalization: rowsum rows -> PE-transpose -> DVE reciprocal ->
    PE-transpose back -> broadcast over dh via a select-row matmul ->
    in-place DVE multiply on O^T.
  - out[t, D] = sum_h matmul(lhsT=O^T_h slice, rhs=Wp_h), PSUM-accumulated.
"""

import numpy as np
from contextlib import ExitStack

import concourse.bass as bass
import concourse.tile as tile
from concourse import mybir
from concourse.bass_utils import run_bass_kernel_spmd
from concourse.vector_clock import ScopedClock
from concourse.masks import make_identity

B, N, M, D = 2, 2048, 2048, 1024
H = 16
DH = D // H  # 64
SCALE = DH ** -0.5
NCORES = 8
HG = 4            # heads per core
CSL = HG * DH     # 256 columns of Wq/Wk/Wv per core
F32 = mybir.dt.float32
F16 = mybir.dt.float16

# ---------------------------------------------------------------------------
# walrus in this container rejects >1 sem wait per instruction; spread the
# extras across preceding same-engine NOPs (queues execute in order, so this
# is semantically identical).
_MAX_WAITS = 1


def _patched_drain_and_barrier(self, tick_clock, wait_clock):
    drain_inst = self.nc.sync.drain()
    wait_clock.add_sem_waits(
        drain_inst.ins, ScopedClock({None: tick_clock.global_clock})
    )
    si = drain_inst.ins.sync_info
    waits = list(si.on_wait or []) if si else []
    if len(waits) > _MAX_WAITS:
        si.on_wait = waits[:_MAX_WAITS]
        for i in range(_MAX_WAITS, len(waits), _MAX_WAITS):
            extra = self.nc.sync.drain()
            extra.ins.sync_info = mybir.SyncInfo(
                on_wait=waits[i : i + _MAX_WAITS], on_update=[]
            )
    self.nc.all_engine_barrier()
    assert self.sems is not None
    popped = self.nc._tile_sem_poison_stack.pop()
    assert popped is self._sem_poison
    self.nc.clear_and_free_semaphores(list(self.sems.allocated().values()))
    self.nc.all_engine_barrier()


tile.TileContext._drain_and_barrier = _patched_drain_and_barrier
# ---------------------------------------------------------------------------

Exp = mybir.ActivationFunctionType.Exp
Identity = mybir.ActivationFunctionType.Identity


def _split_waits(nc):
    n_split = 0
    for bb in nc.main_func.blocks:
        new_list = []
        for ins in bb.instructions:
            si = ins.sync_info
            if si is not None and si.on_wait and len(si.on_wait) > 1:
                waits = list(si.on_wait)
                for j, w in enumerate(waits[:-1]):
                    nop = mybir.InstNoOp(
                        name=f"{ins.name}-sw{j}",
                        engine=ins.engine,
                        sync_info=mybir.SyncInfo(on_wait=[w], on_update=[]),
                    )
                    new_list.append(nop)
                    n_split += 1
                si.on_wait = [waits[-1]]
            new_list.append(ins)
        bb.instructions = new_list
    return n_split


def build_nc(reps: int = 1) -> bass.Bass:
    nc = bass.Bass()

    xqT = nc.dram_tensor("xqT", [D, N], F16, kind="ExternalInput")
    xkvT = nc.dram_tensor("xkvT", [D, M], F16, kind="ExternalInput")
    maskT = nc.dram_tensor("maskT", [M, N], F16, kind="ExternalInput")
    wq = nc.dram_tensor("wq", [D, CSL], F16, kind="ExternalInput")
    wk = nc.dram_tensor("wk", [D, CSL], F16, kind="ExternalInput")
    wv = nc.dram_tensor("wv", [D, CSL], F16, kind="ExternalInput")
    wp = nc.dram_tensor("wp", [CSL, D], F16, kind="ExternalInput")
    bq2 = nc.dram_tensor("bq2", [128, 2], F32, kind="ExternalInput")
    bk2 = nc.dram_tensor("bk2", [128, 2], F32, kind="ExternalInput")
    bv1 = nc.dram_tensor("bv1", [1, CSL], F16, kind="ExternalInput")
    sel4in = nc.dram_tensor("sel4in", [4, HG * DH], F16, kind="ExternalInput")
    outp = nc.dram_tensor("outp", [N, D], F16, kind="ExternalOutput")
    rscratch = nc.dram_tensor("rscratch", [4, N], F16)

    MT = M // 128   # 16 m-chunks
    NT = N // 128   # 16 t-tiles

    with ExitStack() as ctx:
        tc = ctx.enter_context(tile.TileContext(nc))

        consts = ctx.enter_context(tc.tile_pool(name="consts", bufs=1))
        ident = consts.tile([128, 128], F16)
        make_identity(nc, ident)
        ones_row = consts.tile([1, 128], F16)
        nc.vector.memset(ones_row, 1.0)
        sel4 = consts.tile([4, 4, DH], F16)
        nc.sync.dma_start(out=sel4, in_=sel4in[:, :])
        bq_sb = consts.tile([128, 2], F32)
        nc.sync.dma_start(out=bq_sb, in_=bq2[:, :])
        bk_sb = consts.tile([128, 2], F32)
        nc.sync.dma_start(out=bk_sb, in_=bk2[:, :])
        bv_sb = consts.tile([1, CSL], F16)
        nc.sync.dma_start(out=bv_sb, in_=bv1[:, :])
        wp_sb = consts.tile([128, 2, D], F16)
        for g in range(2):
            nc.sync.dma_start(out=wp_sb[:, g, :], in_=wp[g * 128 : (g + 1) * 128, :])

        persist = ctx.enter_context(tc.tile_pool(name="persist", bufs=1))
        KT = [persist.tile([128, M], F16, tag=f"kt{g}", name=f"kt{g}") for g in range(2)]
        QT = [persist.tile([128, N], F16, tag=f"qt{g}", name=f"qt{g}") for g in range(2)]
        V = persist.tile([128, MT, HG, DH + 1], F16, tag="v")
        OTALL = persist.tile([DH + 1, HG, N], F16, tag="ot")
        OT = [OTALL[:, h, :] for h in range(HG)]
        OTP = [persist.tile([128, N], F16, tag=f"otp{g}", name=f"otp{g}") for g in range(2)]
        rowsums = persist.tile([4, N], F16, tag="rs")
        recrows = persist.tile([4, N], F16, tag="rr")
        recipT = persist.tile([128, 64], F16, tag="rcpt")

        # mask halves ([128, MT, 1024] f16 = 32KB/partition each)
        maskp = ctx.enter_context(tc.tile_pool(name="maskp", bufs=1))

        for _rep in range(reps):
            # ------- staging + projections (KT0,QT0,KT1,QT1 first; V last,
            # it is off the critical path to the first attention block) ------
            with (
                tc.tile_pool(name="xkv", bufs=1) as xkvp,
                tc.tile_pool(name="wkv", bufs=1) as wkvp,
                tc.tile_pool(name="xq", bufs=1) as xqp,
                tc.tile_pool(name="wqp", bufs=1) as wqpool,
                tc.tile_pool(name="pp1", bufs=2, space="PSUM") as pp1,
            ):
                xkv_sb = xkvp.tile([128, 8, M], F16)
                for d in range(8):
                    nc.sync.dma_start(
                        out=xkv_sb[:, d, :], in_=xkvT[d * 128 : (d + 1) * 128, :]
                    )
                wk_sb = wkvp.tile([128, 8, CSL], F16)
                wv_sb = wkvp.tile([128, 8, CSL], F16)
                for d in range(8):
                    nc.sync.dma_start(out=wk_sb[:, d, :], in_=wk[d * 128 : (d + 1) * 128, :])
                    nc.sync.dma_start(out=wv_sb[:, d, :], in_=wv[d * 128 : (d + 1) * 128, :])
                xq_sb = xqp.tile([128, 8, N], F16)
                for d in range(8):
                    nc.sync.dma_start(
                        out=xq_sb[:, d, :], in_=xqT[d * 128 : (d + 1) * 128, :]
                    )
                wq_sb = wqpool.tile([128, 8, CSL], F16)
                for d in range(8):
                    nc.sync.dma_start(out=wq_sb[:, d, :], in_=wq[d * 128 : (d + 1) * 128, :])
                mk_halves = [maskp.tile([128, MT, 1024], F16, tag="mk", name="mk0")]
                for m in range(MT):
                    nc.sync.dma_start(
                        out=mk_halves[0][:, m, :],
                        in_=maskT[m * 128 : (m + 1) * 128, 0:1024],
                    )

                def project(g, w_sb, dst, bias_sb):
                    for ms in range(4):
                        ps = pp1.tile([128, 512], F32, tag="proj", name="ps")
                        for d in range(8):
                            nc.tensor.matmul(
                                ps,
                                w_sb[:, d, g * 128 : (g + 1) * 128],
                                (xkv_sb if w_sb is wk_sb else xq_sb)[
                                    :, d, ms * 512 : (ms + 1) * 512
                                ],
                                start=(d == 0),
                                stop=(d == 7),
                            )
                        # bias-add on DVE: keeps the ACT queue clear so the
                        # first attention exp isn't FIFO-blocked behind these
                        nc.vector.tensor_scalar_add(
                            dst[:, ms * 512 : (ms + 1) * 512],
                            ps,
                            bias_sb[:, g : g + 1],
                        )

                project(0, wk_sb, KT[0], bk_sb)
                project(0, wq_sb, QT[0], bq_sb)

                for mt in range(MT):
                    vp = pp1.tile([128, HG, DH], F32, tag="vproj")
                    for d in range(8):
                        nc.tensor.matmul(
                            vp,
                            xkv_sb[:, d, mt * 128 : (mt + 1) * 128],
                            wv_sb[:, d, :],
                            start=(d == 0),
                            stop=False,
                        )
                    nc.tensor.matmul(vp, ones_row, bv_sb, start=False, stop=True)
                    nc.vector.tensor_copy(V[:, mt, :, 0:DH], vp)
                nc.vector.memset(V[:, :, :, DH : DH + 1], 1.0)

                project(1, wk_sb, KT[1], bk_sb)
                project(1, wq_sb, QT[1], bq_sb)

            # ---------------- phase 3: attention ----------------------------
            with (
                tc.tile_pool(name="et", bufs=4) as etp,
                tc.tile_pool(name="otp", bufs=1, space="PSUM") as otpp,
                tc.tile_pool(name="stp", bufs=1, space="PSUM") as stp,
            ):
                for nh in range(2):
                    nhs = nh * 1024
                    if nh == 1:
                        mk = maskp.tile([128, MT, 1024], F16, tag="mk", name="mk1")
                        for m in range(MT):
                            nc.sync.dma_start(
                                out=mk[:, m, :],
                                in_=maskT[m * 128 : (m + 1) * 128, 1024:2048],
                            )
                    else:
                        mk = mk_halves[0]
                    for g in range(2):
                        ota = otpp.tile([DH + 1, 1024], F32, tag="ota")
                        otb = otpp.tile([DH + 1, 1024], F32, tag="otb")
                        for m in range(MT):
                            sta = stp.tile([128, 1024], F32, tag="sta")
                            stb = stp.tile([128, 1024], F32, tag="stb")
                            for ns in range(2):
                                nc.tensor.matmul(
                                    sta[:, ns * 512 : (ns + 1) * 512],
                                    KT[g][0:64, m * 128 : (m + 1) * 128],
                                    QT[g][0:64, nhs + ns * 512 : nhs + (ns + 1) * 512],
                                )
                                nc.tensor.matmul(
                                    stb[:, ns * 512 : (ns + 1) * 512],
                                    KT[g][64:128, m * 128 : (m + 1) * 128],
                                    QT[g][64:128, nhs + ns * 512 : nhs + (ns + 1) * 512],
                                )
                            eta = etp.tile([128, 1024], F16, tag="eta")
                            etb = etp.tile([128, 1024], F16, tag="etb")
                            nc.scalar.activation(eta, sta, Exp, scale=SCALE)
                            nc.scalar.activation(etb, stb, Exp, scale=SCALE)
                            nc.vector.tensor_mul(eta, eta, mk[:, m, :])
                            nc.vector.tensor_mul(etb, etb, mk[:, m, :])
                            for ns in range(2):
                                nc.tensor.matmul(
                                    ota[:, ns * 512 : (ns + 1) * 512],
                                    V[:, m, 2 * g, :],
                                    eta[:, ns * 512 : (ns + 1) * 512],
                                    start=(m == 0),
                                    stop=(m == MT - 1),
                                )
                                nc.tensor.matmul(
                                    otb[:, ns * 512 : (ns + 1) * 512],
                                    V[:, m, 2 * g + 1, :],
                                    etb[:, ns * 512 : (ns + 1) * 512],
                                    start=(m == 0),
                                    stop=(m == MT - 1),
                                )
                        nc.vector.tensor_copy(OT[2 * g][:, nhs : nhs + 1024], ota)
                        nc.vector.tensor_copy(OT[2 * g + 1][:, nhs : nhs + 1024], otb)

            # rowsum rows -> one [4, N] tile (partition shift => DMA)
            nc.sync.dma_start(
                out=rowsums, in_=OTALL[DH : DH + 1, :, :]
            )

            # ---------------- phase 4: normalize ----------------------------
            with tc.tile_pool(name="np1", bufs=1, space="PSUM") as np1:
                rsT = np1.tile([128, 64], F16, tag="rst")
                for b in range(16):
                    nc.tensor.transpose(
                        rsT[:, 4 * b : 4 * b + 4],
                        rowsums[:, b * 128 : (b + 1) * 128],
                        ident[0:4, 0:4],
                    )
                with nc.allow_low_precision(
                    reason="softmax denominators are O(1e3); f16 recip gives "
                    "~5e-4 rel err, well inside the output tolerance"
                ):
                    nc.vector.reciprocal(recipT, rsT)
                rrps = np1.tile([4, N], F16, tag="rrps")
                for b in range(16):
                    nc.tensor.transpose(
                        rrps[:, b * 128 : (b + 1) * 128],
                        recipT[:, 4 * b : 4 * b + 4],
                        ident,
                    )
                nc.vector.tensor_copy(recrows, rrps)

            with (
                tc.tile_pool(name="np2", bufs=2, space="PSUM") as np2,
                tc.tile_pool(name="rsb16", bufs=2) as rsb16p,
            ):
                for h in range(HG):
                    g, sub = divmod(h, 2)
                    rps = np2.tile([DH, N], F32, tag="rbc")
                    for ns in range(4):
                        nc.tensor.matmul(
                            rps[:, ns * 512 : (ns + 1) * 512],
                            sel4[:, h, :],
                            recrows[:, ns * 512 : (ns + 1) * 512],
                        )
                    r16 = rsb16p.tile([DH, N], F16, tag="r16")
                    nc.scalar.copy(r16, rps)
                    if sub == 0:
                        nc.vector.tensor_mul(OTP[g][0:DH, :], OT[h][0:DH, :], r16)
                    else:
                        nc.vector.tensor_mul(OT[h][0:DH, :], OT[h][0:DH, :], r16)
                        # partition shift 0-63 -> 64-127 (only DMA can do this)
                        nc.sync.dma_start(
                            out=OTP[g][DH:128, :], in_=OT[h][0:DH, :]
                        )

            # ---------------- phase 5: output projection --------------------
            with (
                tc.tile_pool(name="ops", bufs=4, space="PSUM") as opp,
                tc.tile_pool(name="osb", bufs=6) as osb,
            ):
                for t in range(NT):
                    po = opp.tile([128, D], F32, tag="po")
                    for g in range(2):
                        for ns in range(2):
                            nc.tensor.matmul(
                                po[:, ns * 512 : (ns + 1) * 512],
                                OTP[g][:, t * 128 : (t + 1) * 128],
                                wp_sb[:, g, ns * 512 : (ns + 1) * 512],
                                start=(g == 0),
                                stop=(g == 1),
                            )
                    ob = osb.tile([128, D], F16, tag="ob")
                    if t % 2 == 0:
                        nc.scalar.copy(ob, po)
                    else:
                        nc.vector.tensor_copy(ob, po)
                    nc.sync.dma_start(out=outp[t * 128 : (t + 1) * 128, :], in_=ob)

    _split_waits(nc)
    return nc


_SEL4 = np.zeros((4, 4, DH), dtype=np.float16)
for _h in range(4):
    _SEL4[_h, _h, :] = 1.0
_SEL4 = np.ascontiguousarray(_SEL4.reshape(4, 4 * DH))

_NC_CACHE = {}
_TRACE = False
_LAST_EXEC_NS = None


def _get_nc():
    if "nc" not in _NC_CACHE:
        _NC_CACHE["nc"] = build_nc()
    return _NC_CACHE["nc"]


def kernel(
    inputs_kv, inputs_q, attention_mask, Wq, bq, Wk, bk, Wv, bv, Wp, bp, **_unused
):
    inputs_kv = np.asarray(inputs_kv, dtype=np.float32)
    inputs_q = np.asarray(inputs_q, dtype=np.float32)
    attention_mask = np.asarray(attention_mask)
    Wq = np.asarray(Wq, dtype=np.float32)
    Wk = np.asarray(Wk, dtype=np.float32)
    Wv = np.asarray(Wv, dtype=np.float32)
    Wp = np.asarray(Wp, dtype=np.float32)
    bq = np.asarray(bq, dtype=np.float32)
    bk = np.asarray(bk, dtype=np.float32)
    bv = np.asarray(bv, dtype=np.float32)
    bp = np.asarray(bp, dtype=np.float32)

    in_maps = []
    for c in range(NCORES):
        bidx, g = divmod(c, HG)
        cs = slice(g * CSL, (g + 1) * CSL)
        in_maps.append(
            {
                "xqT": np.ascontiguousarray(inputs_q[bidx].T.astype(np.float16)),
                "xkvT": np.ascontiguousarray(inputs_kv[bidx].T.astype(np.float16)),
                "maskT": np.ascontiguousarray(
                    attention_mask[bidx, 0].T.astype(np.float16)
                ),
                "wq": np.ascontiguousarray(Wq[:, cs].astype(np.float16)),
                "wk": np.ascontiguousarray(Wk[:, cs].astype(np.float16)),
                "wv": np.ascontiguousarray(Wv[:, cs].astype(np.float16)),
                "wp": np.ascontiguousarray(Wp[cs, :].astype(np.float16)),
                "bq2": np.ascontiguousarray(bq[cs].reshape(2, 128).T),
                "bk2": np.ascontiguousarray(bk[cs].reshape(2, 128).T),
                "bv1": np.ascontiguousarray(bv[cs].reshape(1, CSL).astype(np.float16)),
                "sel4in": _SEL4,
            }
        )

    nc = _get_nc()
    res = run_bass_kernel_spmd(
        nc, in_maps, core_ids=list(range(NCORES)), trace=_TRACE
    )
    global _LAST_EXEC_NS
    _LAST_EXEC_NS = res.exec_time_ns

    out = np.zeros((B, N, D), dtype=np.float32)
    for c in range(NCORES):
        bidx = c // HG
        out[bidx] += res.results[c]["outp"].astype(np.float32)
    out += bp
    return out



# revision 2
# speedup vs baseline: 21.3570x; 21.3570x over previous
"""Trainium2 Bass kernel for nn_MultiHeadAttention (B=2, N=M=2048, D=1024, H=16).

Sharding: 8 cores = 2 batches x 4 head-groups (4 heads per core, tensor-parallel
over the head dim of Wq/Wk/Wv/Wp).  Each core computes a partial output
projection [N, D]; the host sums the 4 partials per batch and adds bp.

v2 dataflow (ACT-saturating pipeline; ACT's exp of the 4 x 2048 x 2048 score
matrix at 1 elem/lane/cycle is the hard floor, ~128us/core):
  - prelude: project K^T, V (all m), Q^T (n-strip 0).  f16 operands, PE
    contracts over partitions, DVE does the bias-add evacuations.
  - attention runs over 4 n-strips of 512.  Per (strip, g, m-chunk):
    two 64-contraction score matmuls (heads 2g / 2g+1, PE row-tiles at
    partition bases 0/64 run them concurrently) into a double-buffered
    [128, 2, 512] PSUM pair; ONE ACT exp op over the full [128, 1024] with
    the 1/sqrt(dh) scale folded in; two DVE f16 mask multiplies; two AV
    matmuls accumulating [65, 512] per head (ones-column = softmax denom).
  - per-strip tail (overlaps next strip's attention): evacuate AV, gather
    denominators (tiny shift-DMA), PE-transpose -> DVE f16 reciprocal ->
    transpose back, per-head broadcast via sel4 matmul, normalize in-place,
    then the output projection (4 x K=64 matmuls per [128,512] out tile)
    with evacuation alternating ACT/DVE, and the Q^T projection for strip
    s+1 slotted between the two g-groups.
"""

import numpy as np
from contextlib import ExitStack

import concourse.bass as bass
import concourse.tile as tile
from concourse import mybir
from concourse.bass_utils import run_bass_kernel_spmd
from concourse.vector_clock import ScopedClock
from concourse.masks import make_identity

B, N, M, D = 2, 2048, 2048, 1024
H = 16
DH = D // H  # 64
SCALE = DH ** -0.5
NCORES = 8
HG = 4            # heads per core
CSL = HG * DH     # 256 columns of Wq/Wk/Wv per core
F32 = mybir.dt.float32
F16 = mybir.dt.float16

NSTRIP = 512          # attention n-strip width
NS = N // NSTRIP      # 4 strips
MT = M // 128         # 16 m-chunks

# ---------------------------------------------------------------------------
# walrus in this container rejects >1 sem wait per instruction; spread the
# extras across preceding same-engine NOPs (queues execute in order, so this
# is semantically identical).
_MAX_WAITS = 1


def _patched_drain_and_barrier(self, tick_clock, wait_clock):
    drain_inst = self.nc.sync.drain()
    wait_clock.add_sem_waits(
        drain_inst.ins, ScopedClock({None: tick_clock.global_clock})
    )
    si = drain_inst.ins.sync_info
    waits = list(si.on_wait or []) if si else []
    if len(waits) > _MAX_WAITS:
        si.on_wait = waits[:_MAX_WAITS]
        for i in range(_MAX_WAITS, len(waits), _MAX_WAITS):
            extra = self.nc.sync.drain()
            extra.ins.sync_info = mybir.SyncInfo(
                on_wait=waits[i : i + _MAX_WAITS], on_update=[]
            )
    self.nc.all_engine_barrier()
    assert self.sems is not None
    popped = self.nc._tile_sem_poison_stack.pop()
    assert popped is self._sem_poison
    self.nc.clear_and_free_semaphores(list(self.sems.allocated().values()))
    self.nc.all_engine_barrier()


tile.TileContext._drain_and_barrier = _patched_drain_and_barrier
# ---------------------------------------------------------------------------

Exp = mybir.ActivationFunctionType.Exp


def _split_waits(nc):
    n_split = 0
    for bb in nc.main_func.blocks:
        new_list = []
        for ins in bb.instructions:
            si = ins.sync_info
            if si is not None and si.on_wait and len(si.on_wait) > 1:
                waits = list(si.on_wait)
                for j, w in enumerate(waits[:-1]):
                    nop = mybir.InstNoOp(
                        name=f"{ins.name}-sw{j}",
                        engine=ins.engine,
                        sync_info=mybir.SyncInfo(on_wait=[w], on_update=[]),
                    )
                    new_list.append(nop)
                    n_split += 1
                si.on_wait = [waits[-1]]
            new_list.append(ins)
        bb.instructions = new_list
    return n_split


def build_nc(reps: int = 1) -> bass.Bass:
    nc = bass.Bass()

    xqT = nc.dram_tensor("xqT", [D, N], F16, kind="ExternalInput")
    xkvT = nc.dram_tensor("xkvT", [D, M], F16, kind="ExternalInput")
    maskT = nc.dram_tensor("maskT", [M, N], F16, kind="ExternalInput")
    wq = nc.dram_tensor("wq", [D, CSL], F16, kind="ExternalInput")
    wk = nc.dram_tensor("wk", [D, CSL], F16, kind="ExternalInput")
    wv = nc.dram_tensor("wv", [D, CSL], F16, kind="ExternalInput")
    wp = nc.dram_tensor("wp", [CSL, D], F16, kind="ExternalInput")
    bq2 = nc.dram_tensor("bq2", [128, 2], F32, kind="ExternalInput")
    bk2 = nc.dram_tensor("bk2", [128, 2], F32, kind="ExternalInput")
    bv1 = nc.dram_tensor("bv1", [1, CSL], F16, kind="ExternalInput")
    sel4in = nc.dram_tensor("sel4in", [4, HG * DH], F16, kind="ExternalInput")
    outp = nc.dram_tensor("outp", [N, D], F16, kind="ExternalOutput")

    with ExitStack() as ctx:
        tc = ctx.enter_context(tile.TileContext(nc))

        consts = ctx.enter_context(tc.tile_pool(name="consts", bufs=1))
        ident = consts.tile([128, 128], F16)
        make_identity(nc, ident)
        ones_row = consts.tile([1, 128], F16)
        nc.vector.memset(ones_row, 1.0)
        sel4 = consts.tile([4, 4, DH], F16)
        nc.sync.dma_start(out=sel4, in_=sel4in[:, :])
        bq_sb = consts.tile([128, 2], F32)
        nc.sync.dma_start(out=bq_sb, in_=bq2[:, :])
        bk_sb = consts.tile([128, 2], F32)
        nc.sync.dma_start(out=bk_sb, in_=bk2[:, :])
        bv_sb = consts.tile([1, CSL], F16)
        nc.sync.dma_start(out=bv_sb, in_=bv1[:, :])
        # wp per head at partitions 0-63: [64, h, slice, 512]
        wp_sb = consts.tile([DH, HG, 2, 512], F16)
        for h in range(HG):
            nc.sync.dma_start(
                out=wp_sb[:, h, :, :], in_=wp[h * DH : (h + 1) * DH, :]
            )

        persist = ctx.enter_context(tc.tile_pool(name="persist", bufs=1))
        KT = [persist.tile([128, M], F16, tag=f"kt{g}", name=f"kt{g}") for g in range(2)]
        QT = [persist.tile([128, N], F16, tag=f"qt{g}", name=f"qt{g}") for g in range(2)]
        V = persist.tile([128, MT, HG, DH + 1], F16, tag="v")
        wq_sb = persist.tile([128, 8, CSL], F16, tag="wq")
        for d in range(8):
            nc.sync.dma_start(out=wq_sb[:, d, :], in_=wq[d * 128 : (d + 1) * 128, :])

        # rotating pools that live across the rep body
        maskp = ctx.enter_context(tc.tile_pool(name="maskp", bufs=1))
        xqp = ctx.enter_context(tc.tile_pool(name="xqp", bufs=2))
        etap = ctx.enter_context(tc.tile_pool(name="etap", bufs=4))
        otsp = ctx.enter_context(tc.tile_pool(name="otsp", bufs=2))
        obp = ctx.enter_context(tc.tile_pool(name="obp", bufs=2))
        nrmp = ctx.enter_context(tc.tile_pool(name="nrmp", bufs=2))

        def dma_xq_strip(s):
            xq_sb = xqp.tile([128, 8, NSTRIP], F16, tag="xq", name=f"xq{s}")
            for d in range(8):
                nc.sync.dma_start(
                    out=xq_sb[:, d, :],
                    in_=xqT[d * 128 : (d + 1) * 128, s * NSTRIP : (s + 1) * NSTRIP],
                )
            return xq_sb

        def dma_mask_half(nh):
            mk = maskp.tile([128, MT, 1024], F16, tag="mk", name=f"mk{nh}")
            for m in range(MT):
                nc.sync.dma_start(
                    out=mk[:, m, :],
                    in_=maskT[m * 128 : (m + 1) * 128, nh * 1024 : (nh + 1) * 1024],
                )
            return mk

        def project_q(pool, xq_sb, s):
            """Q^T for strip s into QT[g][:, s*NSTRIP...], via pool psum."""
            for g in range(2):
                ps = pool.tile([128, NSTRIP], F32, tag="t", name=f"qps{s}{g}")
                for d in range(8):
                    nc.tensor.matmul(
                        ps,
                        wq_sb[:, d, g * 128 : (g + 1) * 128],
                        xq_sb[:, d, :],
                        start=(d == 0),
                        stop=(d == 7),
                    )
                nc.vector.tensor_scalar_add(
                    QT[g][:, s * NSTRIP : (s + 1) * NSTRIP], ps, bq_sb[:, g : g + 1]
                )

        for _rep in range(reps):
            # ---------------- phase 1: K/V projections + Q strip 0 ----------
            with (
                tc.tile_pool(name="xkv", bufs=2) as xkvp,
                tc.tile_pool(name="wkv", bufs=1) as wkvp,
                tc.tile_pool(name="pp1", bufs=4, space="PSUM") as pp1,
            ):
                xq0 = dma_xq_strip(0)
                wk_sb = wkvp.tile([128, 8, CSL], F16)
                wv_sb = wkvp.tile([128, 8, CSL], F16)
                for d in range(8):
                    nc.sync.dma_start(out=wk_sb[:, d, :], in_=wk[d * 128 : (d + 1) * 128, :])
                    nc.sync.dma_start(out=wv_sb[:, d, :], in_=wv[d * 128 : (d + 1) * 128, :])

                mk_cur = dma_mask_half(0)

                # Q strip 0 first (wq/xq0 are small DMAs; PE starts earliest)
                project_q(pp1, xq0, 0)

                for mh in range(2):  # m-halves of 1024
                    xkv_sb = xkvp.tile([128, 8, 1024], F16, tag="xkv", name=f"xkv{mh}")
                    for d in range(8):
                        nc.sync.dma_start(
                            out=xkv_sb[:, d, :],
                            in_=xkvT[d * 128 : (d + 1) * 128, mh * 1024 : (mh + 1) * 1024],
                        )
                    # K^T for this m-half
                    for g in range(2):
                        for ms in range(2):
                            ps = pp1.tile([128, 512], F32, tag="t", name="kps")
                            for d in range(8):
                                nc.tensor.matmul(
                                    ps,
                                    wk_sb[:, d, g * 128 : (g + 1) * 128],
                                    xkv_sb[:, d, ms * 512 : (ms + 1) * 512],
                                    start=(d == 0),
                                    stop=(d == 7),
                                )
                            nc.vector.tensor_scalar_add(
                                KT[g][:, mh * 1024 + ms * 512 : mh * 1024 + (ms + 1) * 512],
                                ps,
                                bk_sb[:, g : g + 1],
                            )
                    # V for this m-half
                    for mt in range(8):
                        vp = pp1.tile([128, HG, DH], F32, tag="vp", bufs=2, name="vp")
                        for d in range(8):
                            nc.tensor.matmul(
                                vp,
                                xkv_sb[:, d, mt * 128 : (mt + 1) * 128],
                                wv_sb[:, d, :],
                                start=(d == 0),
                                stop=False,
                            )
                        nc.tensor.matmul(vp, ones_row, bv_sb, start=False, stop=True)
                        nc.vector.tensor_copy(V[:, mh * 8 + mt, :, 0:DH], vp)
                nc.vector.memset(V[:, :, :, DH : DH + 1], 1.0)

            # ---------------- phase 2: attention over n-strips --------------
            with (
                tc.tile_pool(name="stp", bufs=2, space="PSUM") as stp,
                tc.tile_pool(name="otp", bufs=1, space="PSUM") as otp,
                tc.tile_pool(name="tailp", bufs=2, space="PSUM") as tailp,
            ):
                for s in range(NS):
                    nh = s // 2
                    if s == 1:
                        mk_nxt = dma_mask_half(1)
                    no = (s % 2) * 512  # n-offset inside the mask half
                    mk = mk_cur if s < 2 else mk_nxt

                    ots = otsp.tile([DH + 1, HG, NSTRIP], F16, tag="ots", name=f"ots{s}")
                    for g in range(2):
                        otg = otp.tile([DH + 1, 2, NSTRIP], F32, tag="ot", name=f"ot{s}{g}")
                        for m in range(MT):
                            s2 = stp.tile([128, 2, NSTRIP], F32, tag="s2", name="s2")
                            for i in range(2):
                                nc.tensor.matmul(
                                    s2[:, i, :],
                                    KT[g][i * 64 : (i + 1) * 64, m * 128 : (m + 1) * 128],
                                    QT[g][
                                        i * 64 : (i + 1) * 64,
                                        s * NSTRIP : (s + 1) * NSTRIP,
                                    ],
                                )
                            eta = etap.tile([128, 2, NSTRIP], F16, tag="eta", name="eta")
                            nc.scalar.activation(eta, s2, Exp, scale=SCALE)
                            for i in range(2):
                                nc.vector.tensor_mul(
                                    eta[:, i, :], eta[:, i, :], mk[:, m, no : no + 512]
                                )
                            for i in range(2):
                                nc.tensor.matmul(
                                    otg[:, i, :],
                                    V[:, m, 2 * g + i, :],
                                    eta[:, i, :],
                                    start=(m == 0),
                                    stop=(m == MT - 1),
                                )
                        for i in range(2):
                            nc.vector.tensor_copy(
                                ots[:, 2 * g + i, :], otg[0 : DH + 1, i, :]
                            )
                        if g == 0:
                            # overlap: Q^T projection for strip s+1
                            if s + 1 < NS:
                                xq_nxt = dma_xq_strip(s + 1)
                                project_q(tailp, xq_nxt, s + 1)

                    # ---- strip tail: normalize + output projection ----
                    rowsums = nrmp.tile([4, NSTRIP], F16, tag="rs", name="rs")
                    nc.sync.dma_start(out=rowsums, in_=ots[DH : DH + 1, :, :])
                    rsT = tailp.tile([128, 16], F16, tag="t", name="rsT")
                    for b in range(4):
                        nc.tensor.transpose(
                            rsT[:, 4 * b : 4 * b + 4],
                            rowsums[:, b * 128 : (b + 1) * 128],
                            ident[0:4, 0:4],
                        )
                    recipT = nrmp.tile([128, 16], F16, tag="rcp", name="recipT")
                    with nc.allow_low_precision(
                        reason="softmax denominators are O(1e3); f16 recip gives "
                        "~5e-4 rel err, well inside the output tolerance"
                    ):
                        nc.vector.reciprocal(recipT, rsT)
                    rrps = tailp.tile([4, NSTRIP], F16, tag="t", name="rrps")
                    for b in range(4):
                        nc.tensor.transpose(
                            rrps[:, b * 128 : (b + 1) * 128],
                            recipT[:, 4 * b : 4 * b + 4],
                            ident,
                        )
                    recrows = nrmp.tile([4, NSTRIP], F16, tag="rr", name="recrows")
                    nc.vector.tensor_copy(recrows, rrps)

                    for h in range(HG):
                        rps = tailp.tile([DH, NSTRIP], F32, tag="t", name="rps")
                        nc.tensor.matmul(rps, sel4[:, h, :], recrows)
                        nc.vector.tensor_mul(
                            ots[0:DH, h, :], ots[0:DH, h, :], rps
                        )

                    for t in range(NSTRIP // 128):
                        ob = obp.tile([128, 2, 512], F16, tag="ob", name="ob")
                        for sl in range(2):
                            po = tailp.tile([128, 512], F32, tag="t", name="po")
                            for h in range(HG):
                                nc.tensor.matmul(
                                    po,
                                    ots[0:DH, h, t * 128 : (t + 1) * 128],
                                    wp_sb[:, h, sl, :],
                                    start=(h == 0),
                                    stop=(h == HG - 1),
                                )
                            if (t + sl) % 2 == 0:
                                nc.scalar.copy(ob[:, sl, :], po)
                            else:
                                nc.vector.tensor_copy(ob[:, sl, :], po)
                        row = s * NSTRIP + t * 128
                        nc.sync.dma_start(out=outp[row : row + 128, :], in_=ob)

                    if s >= 2:
                        mk_cur = mk_nxt

    _split_waits(nc)
    return nc


_SEL4 = np.zeros((4, 4, DH), dtype=np.float16)
for _h in range(4):
    _SEL4[_h, _h, :] = 1.0
_SEL4 = np.ascontiguousarray(_SEL4.reshape(4, 4 * DH))

_NC_CACHE = {}
_TRACE = False
_LAST_EXEC_NS = None


def _get_nc():
    if "nc" not in _NC_CACHE:
        _NC_CACHE["nc"] = build_nc()
    return _NC_CACHE["nc"]


def make_in_maps(inputs_q, inputs_kv, attention_mask, Wq, bq, Wk, bk, Wv, bv, Wp):
    """Per-core input dicts (shared by kernel() and test.py's bench)."""
    in_maps = []
    for c in range(NCORES):
        bidx, g = divmod(c, HG)
        cs = slice(g * CSL, (g + 1) * CSL)
        in_maps.append(
            {
                "xqT": np.ascontiguousarray(inputs_q[bidx].T.astype(np.float16)),
                "xkvT": np.ascontiguousarray(inputs_kv[bidx].T.astype(np.float16)),
                "maskT": np.ascontiguousarray(
                    attention_mask[bidx, 0].T.astype(np.float16)
                ),
                "wq": np.ascontiguousarray(Wq[:, cs].astype(np.float16)),
                "wk": np.ascontiguousarray(Wk[:, cs].astype(np.float16)),
                "wv": np.ascontiguousarray(Wv[:, cs].astype(np.float16)),
                "wp": np.ascontiguousarray(Wp[cs, :].astype(np.float16)),
                "bq2": np.ascontiguousarray(
                    bq[cs].astype(np.float32).reshape(2, 128).T
                ),
                "bk2": np.ascontiguousarray(
                    bk[cs].astype(np.float32).reshape(2, 128).T
                ),
                "bv1": np.ascontiguousarray(bv[cs].reshape(1, CSL).astype(np.float16)),
                "sel4in": _SEL4,
            }
        )
    return in_maps


def kernel(
    inputs_kv, inputs_q, attention_mask, Wq, bq, Wk, bk, Wv, bv, Wp, bp, **_unused
):
    inputs_kv = np.asarray(inputs_kv, dtype=np.float32)
    inputs_q = np.asarray(inputs_q, dtype=np.float32)
    attention_mask = np.asarray(attention_mask)
    bp = np.asarray(bp, dtype=np.float32)

    in_maps = make_in_maps(
        inputs_q, inputs_kv, attention_mask,
        np.asarray(Wq, dtype=np.float32), np.asarray(bq, dtype=np.float32),
        np.asarray(Wk, dtype=np.float32), np.asarray(bk, dtype=np.float32),
        np.asarray(Wv, dtype=np.float32), np.asarray(bv, dtype=np.float32),
        np.asarray(Wp, dtype=np.float32),
    )

    nc = _get_nc()
    res = run_bass_kernel_spmd(
        nc, in_maps, core_ids=list(range(NCORES)), trace=_TRACE
    )
    global _LAST_EXEC_NS
    _LAST_EXEC_NS = res.exec_time_ns

    out = np.zeros((B, N, D), dtype=np.float32)
    for c in range(NCORES):
        bidx = c // HG
        out[bidx] += res.results[c]["outp"].astype(np.float32)
    out += bp
    return out


# revision 4
# speedup vs baseline: 1634.3796x; 76.5267x over previous
"""Trainium2 Bass kernel for nn_MultiHeadAttention (B=2, N=M=2048, D=1024, H=16).

Sharding: 8 cores = 2 batches x 4 head-groups (4 heads per core, tensor-parallel
over the head dim of Wq/Wk/Wv/Wp).  Each core computes a partial output
projection [N, D]; the host sums the 4 partials per batch and adds bp.

v2 dataflow (ACT-saturating pipeline; ACT's exp of the 4 x 2048 x 2048 score
matrix at 1 elem/lane/cycle is the hard floor, ~128us/core):
  - prelude: project K^T, V (all m), Q^T (n-strip 0).  f16 operands, PE
    contracts over partitions, DVE does the bias-add evacuations.
  - attention runs over 4 n-strips of 512.  Per (strip, g, m-chunk):
    two 64-contraction score matmuls (heads 2g / 2g+1, PE row-tiles at
    partition bases 0/64 run them concurrently) into a double-buffered
    [128, 2, 512] PSUM pair; ONE ACT exp op over the full [128, 1024] with
    the 1/sqrt(dh) scale folded in; two DVE f16 mask multiplies; two AV
    matmuls accumulating [65, 512] per head (ones-column = softmax denom).
  - per-strip tail (overlaps next strip's attention): evacuate AV, gather
    denominators (tiny shift-DMA), PE-transpose -> DVE f16 reciprocal ->
    transpose back, per-head broadcast via sel4 matmul, normalize in-place,
    then the output projection (4 x K=64 matmuls per [128,512] out tile)
    with evacuation alternating ACT/DVE, and the Q^T projection for strip
    s+1 slotted between the two g-groups.
"""

import numpy as np
from contextlib import ExitStack

import concourse.bass as bass
import concourse.tile as tile
from concourse import mybir
from concourse.bass_utils import run_bass_kernel_spmd
from concourse.vector_clock import ScopedClock
from concourse.masks import make_identity

B, N, M, D = 2, 2048, 2048, 1024
H = 16
DH = D // H  # 64
SCALE = DH ** -0.5
NCORES = 8
HG = 4            # heads per core
CSL = HG * DH     # 256 columns of Wq/Wk/Wv per core
F32 = mybir.dt.float32
F16 = mybir.dt.float16

NSTRIP = 512          # attention n-strip width
NS = N // NSTRIP      # 4 strips
MT = M // 128         # 16 m-chunks

# ---------------------------------------------------------------------------
# walrus in this container rejects >1 sem wait per instruction; spread the
# extras across preceding same-engine NOPs (queues execute in order, so this
# is semantically identical).
_MAX_WAITS = 1


def _patched_drain_and_barrier(self, tick_clock, wait_clock):
    drain_inst = self.nc.sync.drain()
    wait_clock.add_sem_waits(
        drain_inst.ins, ScopedClock({None: tick_clock.global_clock})
    )
    si = drain_inst.ins.sync_info
    waits = list(si.on_wait or []) if si else []
    if len(waits) > _MAX_WAITS:
        si.on_wait = waits[:_MAX_WAITS]
        for i in range(_MAX_WAITS, len(waits), _MAX_WAITS):
            extra = self.nc.sync.drain()
            extra.ins.sync_info = mybir.SyncInfo(
                on_wait=waits[i : i + _MAX_WAITS], on_update=[]
            )
    self.nc.all_engine_barrier()
    assert self.sems is not None
    popped = self.nc._tile_sem_poison_stack.pop()
    assert popped is self._sem_poison
    self.nc.clear_and_free_semaphores(list(self.sems.allocated().values()))
    self.nc.all_engine_barrier()


tile.TileContext._drain_and_barrier = _patched_drain_and_barrier
# ---------------------------------------------------------------------------

Exp = mybir.ActivationFunctionType.Exp


def _split_waits(nc):
    n_split = 0
    for bb in nc.main_func.blocks:
        new_list = []
        for ins in bb.instructions:
            si = ins.sync_info
            if si is not None and si.on_wait and len(si.on_wait) > 1:
                waits = list(si.on_wait)
                for j, w in enumerate(waits[:-1]):
                    nop = mybir.InstNoOp(
                        name=f"{ins.name}-sw{j}",
                        engine=ins.engine,
                        sync_info=mybir.SyncInfo(on_wait=[w], on_update=[]),
                    )
                    new_list.append(nop)
                    n_split += 1
                si.on_wait = [waits[-1]]
            new_list.append(ins)
        bb.instructions = new_list
    return n_split


def build_nc(reps: int = 1) -> bass.Bass:
    nc = bass.Bass()

    xqT = nc.dram_tensor("xqT", [D, N], F16, kind="ExternalInput")
    xkvT = nc.dram_tensor("xkvT", [D, M], F16, kind="ExternalInput")
    maskT = nc.dram_tensor("maskT", [M, N], F16, kind="ExternalInput")
    wq = nc.dram_tensor("wq", [D, CSL], F16, kind="ExternalInput")
    wk = nc.dram_tensor("wk", [D, CSL], F16, kind="ExternalInput")
    wv = nc.dram_tensor("wv", [D, CSL], F16, kind="ExternalInput")
    wp = nc.dram_tensor("wp", [CSL, D], F16, kind="ExternalInput")
    bq2 = nc.dram_tensor("bq2", [128, 2], F32, kind="ExternalInput")
    bk2 = nc.dram_tensor("bk2", [128, 2], F32, kind="ExternalInput")
    bv1 = nc.dram_tensor("bv1", [1, CSL], F16, kind="ExternalInput")
    sel4in = nc.dram_tensor("sel4in", [4, HG * DH], F16, kind="ExternalInput")
    outp = nc.dram_tensor("outp", [N, D], F16, kind="ExternalOutput")

    with ExitStack() as ctx:
        tc = ctx.enter_context(tile.TileContext(nc))

        consts = ctx.enter_context(tc.tile_pool(name="consts", bufs=1))
        ident = consts.tile([128, 128], F16)
        make_identity(nc, ident)
        ones_row = consts.tile([1, 128], F16)
        nc.vector.memset(ones_row, 1.0)
        sel4 = consts.tile([4, 4, DH], F16)
        nc.sync.dma_start(out=sel4, in_=sel4in[:, :])
        bq_sb = consts.tile([128, 2], F32)
        nc.sync.dma_start(out=bq_sb, in_=bq2[:, :])
        bk_sb = consts.tile([128, 2], F32)
        nc.sync.dma_start(out=bk_sb, in_=bk2[:, :])
        bv_sb = consts.tile([1, CSL], F16)
        nc.sync.dma_start(out=bv_sb, in_=bv1[:, :])
        # wp per head at partitions 0-63: [64, h, slice, 512]
        wp_sb = consts.tile([DH, HG, 2, 512], F16)
        for h in range(HG):
            nc.sync.dma_start(
                out=wp_sb[:, h, :, :], in_=wp[h * DH : (h + 1) * DH, :]
            )

        persist = ctx.enter_context(tc.tile_pool(name="persist", bufs=1))
        KT = [persist.tile([128, M], F16, tag=f"kt{g}", name=f"kt{g}") for g in range(2)]
        QT = [persist.tile([128, N], F16, tag=f"qt{g}", name=f"qt{g}") for g in range(2)]
        V = persist.tile([128, MT, HG, DH + 1], F16, tag="v")
        wq_sb = persist.tile([128, 8, CSL], F16, tag="wq")
        for d in range(8):
            nc.sync.dma_start(out=wq_sb[:, d, :], in_=wq[d * 128 : (d + 1) * 128, :])

        # rotating pools that live across the rep body
        maskp = ctx.enter_context(tc.tile_pool(name="maskp", bufs=2))
        xqp = ctx.enter_context(tc.tile_pool(name="xqp", bufs=2))
        etap = ctx.enter_context(tc.tile_pool(name="etap", bufs=4))
        otsp = ctx.enter_context(tc.tile_pool(name="otsp", bufs=2))
        obp = ctx.enter_context(tc.tile_pool(name="obp", bufs=2))
        nrmp = ctx.enter_context(tc.tile_pool(name="nrmp", bufs=2))

        def dma_xq_strip(s):
            xq_sb = xqp.tile([128, 8, NSTRIP], F16, tag="xq", name=f"xq{s}")
            for d in range(8):
                nc.sync.dma_start(
                    out=xq_sb[:, d, :],
                    in_=xqT[d * 128 : (d + 1) * 128, s * NSTRIP : (s + 1) * NSTRIP],
                )
            return xq_sb

        def dma_mask_half(nh):
            mk = maskp.tile([128, MT, 1024], F16, tag="mk", name=f"mk{nh}")
            for m in range(MT):
                nc.sync.dma_start(
                    out=mk[:, m, :],
                    in_=maskT[m * 128 : (m + 1) * 128, nh * 1024 : (nh + 1) * 1024],
                )
            return mk

        def project_q(pool, xq_sb, s):
            """Q^T for strip s into QT[g][:, s*NSTRIP...], via pool psum."""
            for g in range(2):
                ps = pool.tile([128, NSTRIP], F32, tag="t", name=f"qps{s}{g}")
                for d in range(8):
                    nc.tensor.matmul(
                        ps,
                        wq_sb[:, d, g * 128 : (g + 1) * 128],
                        xq_sb[:, d, :],
                        start=(d == 0),
                        stop=(d == 7),
                    )
                nc.vector.tensor_scalar_add(
                    QT[g][:, s * NSTRIP : (s + 1) * NSTRIP], ps, bq_sb[:, g : g + 1]
                )

        for _rep in range(reps):
            # ---------------- phase 1: K/V projections + Q strip 0 ----------
            with (
                tc.tile_pool(name="xkv", bufs=2) as xkvp,
                tc.tile_pool(name="wkv", bufs=1) as wkvp,
                tc.tile_pool(name="pp1", bufs=4, space="PSUM") as pp1,
            ):
                xq0 = dma_xq_strip(0)
                wk_sb = wkvp.tile([128, 8, CSL], F16)
                wv_sb = wkvp.tile([128, 8, CSL], F16)
                for d in range(8):
                    nc.sync.dma_start(out=wk_sb[:, d, :], in_=wk[d * 128 : (d + 1) * 128, :])
                    nc.sync.dma_start(out=wv_sb[:, d, :], in_=wv[d * 128 : (d + 1) * 128, :])

                mk_cur = dma_mask_half(0)

                # Q strip 0 first (wq/xq0 are small DMAs; PE starts earliest)
                project_q(pp1, xq0, 0)

                for mh in range(2):  # m-halves of 1024
                    xkv_sb = xkvp.tile([128, 8, 1024], F16, tag="xkv", name=f"xkv{mh}")
                    for d in range(8):
                        nc.sync.dma_start(
                            out=xkv_sb[:, d, :],
                            in_=xkvT[d * 128 : (d + 1) * 128, mh * 1024 : (mh + 1) * 1024],
                        )
                    # K^T for this m-half
                    for g in range(2):
                        for ms in range(2):
                            ps = pp1.tile([128, 512], F32, tag="t", name="kps")
                            for d in range(8):
                                nc.tensor.matmul(
                                    ps,
                                    wk_sb[:, d, g * 128 : (g + 1) * 128],
                                    xkv_sb[:, d, ms * 512 : (ms + 1) * 512],
                                    start=(d == 0),
                                    stop=(d == 7),
                                )
                            nc.vector.tensor_scalar_add(
                                KT[g][:, mh * 1024 + ms * 512 : mh * 1024 + (ms + 1) * 512],
                                ps,
                                bk_sb[:, g : g + 1],
                            )
                    # V for this m-half
                    for mt in range(8):
                        vp = pp1.tile([128, HG, DH], F32, tag="vp", bufs=2, name="vp")
                        for d in range(8):
                            nc.tensor.matmul(
                                vp,
                                xkv_sb[:, d, mt * 128 : (mt + 1) * 128],
                                wv_sb[:, d, :],
                                start=(d == 0),
                                stop=False,
                            )
                        nc.tensor.matmul(vp, ones_row, bv_sb, start=False, stop=True)
                        nc.vector.tensor_copy(V[:, mh * 8 + mt, :, 0:DH], vp)
                nc.vector.memset(V[:, :, :, DH : DH + 1], 1.0)

            # ---------------- phase 2: attention over n-strips --------------
            with (
                tc.tile_pool(name="stp", bufs=2, space="PSUM") as stp,
                tc.tile_pool(name="otp", bufs=1, space="PSUM") as otp,
                tc.tile_pool(name="tailp", bufs=2, space="PSUM") as tailp,
            ):
                for s in range(NS):
                    nh = s // 2
                    if s == 1:
                        mk_nxt = dma_mask_half(1)
                    no = (s % 2) * 512  # n-offset inside the mask half
                    mk = mk_cur if s < 2 else mk_nxt

                    ots = otsp.tile([DH + 1, HG, NSTRIP], F16, tag="ots", name=f"ots{s}")
                    for g in range(2):
                        otg = otp.tile([DH + 1, 2, NSTRIP], F32, tag="ot", name=f"ot{s}{g}")
                        for m in range(MT):
                            s2 = stp.tile([128, 2, NSTRIP], F32, tag="s2", name="s2")
                            for i in range(2):
                                nc.tensor.matmul(
                                    s2[:, i, :],
                                    KT[g][i * 64 : (i + 1) * 64, m * 128 : (m + 1) * 128],
                                    QT[g][
                                        i * 64 : (i + 1) * 64,
                                        s * NSTRIP : (s + 1) * NSTRIP,
                                    ],
                                )
                            eta = etap.tile([128, 2, NSTRIP], F16, tag="eta", name="eta")
                            nc.scalar.activation(eta, s2, Exp, scale=SCALE)
                            for i in range(2):
                                nc.vector.tensor_mul(
                                    eta[:, i, :], eta[:, i, :], mk[:, m, no : no + 512]
                                )
                            for i in range(2):
                                nc.tensor.matmul(
                                    otg[:, i, :],
                                    V[:, m, 2 * g + i, :],
                                    eta[:, i, :],
                                    start=(m == 0),
                                    stop=(m == MT - 1),
                                )
                        for i in range(2):
                            nc.vector.tensor_copy(
                                ots[:, 2 * g + i, :], otg[0 : DH + 1, i, :]
                            )
                        if g == 0:
                            # overlap: Q^T projection for strip s+1
                            if s + 1 < NS:
                                xq_nxt = dma_xq_strip(s + 1)
                                project_q(tailp, xq_nxt, s + 1)

                    # ---- strip tail: normalize + output projection ----
                    rowsums = nrmp.tile([4, NSTRIP], F16, tag="rs", name="rs")
                    nc.sync.dma_start(out=rowsums, in_=ots[DH : DH + 1, :, :])
                    rsT = tailp.tile([128, 16], F16, tag="t", name="rsT")
                    for b in range(4):
                        nc.tensor.transpose(
                            rsT[:, 4 * b : 4 * b + 4],
                            rowsums[:, b * 128 : (b + 1) * 128],
                            ident[0:4, 0:4],
                        )
                    recipT = nrmp.tile([128, 16], F16, tag="rcp", name="recipT")
                    with nc.allow_low_precision(
                        reason="softmax denominators are O(1e3); f16 recip gives "
                        "~5e-4 rel err, well inside the output tolerance"
                    ):
                        nc.vector.reciprocal(recipT, rsT)
                    rrps = tailp.tile([4, NSTRIP], F16, tag="t", name="rrps")
                    for b in range(4):
                        nc.tensor.transpose(
                            rrps[:, b * 128 : (b + 1) * 128],
                            recipT[:, 4 * b : 4 * b + 4],
                            ident,
                        )
                    recrows = nrmp.tile([4, NSTRIP], F16, tag="rr", name="recrows")
                    nc.vector.tensor_copy(recrows, rrps)

                    for h in range(HG):
                        rps = tailp.tile([DH, NSTRIP], F32, tag="t", name="rps")
                        nc.tensor.matmul(rps, sel4[:, h, :], recrows)
                        nc.vector.tensor_mul(
                            ots[0:DH, h, :], ots[0:DH, h, :], rps
                        )

                    for t in range(NSTRIP // 128):
                        ob = obp.tile([128, 2, 512], F16, tag="ob", name="ob")
                        for sl in range(2):
                            po = tailp.tile([128, 512], F32, tag="t", name="po")
                            for h in range(HG):
                                nc.tensor.matmul(
                                    po,
                                    ots[0:DH, h, t * 128 : (t + 1) * 128],
                                    wp_sb[:, h, sl, :],
                                    start=(h == 0),
                                    stop=(h == HG - 1),
                                )
                            if (t + sl) % 2 == 0:
                                nc.scalar.copy(ob[:, sl, :], po)
                            else:
                                nc.vector.tensor_copy(ob[:, sl, :], po)
                        row = s * NSTRIP + t * 128
                        nc.sync.dma_start(out=outp[row : row + 128, :], in_=ob)

    _split_waits(nc)
    return nc


_SEL4 = np.zeros((4, 4, DH), dtype=np.float16)
for _h in range(4):
    _SEL4[_h, _h, :] = 1.0
_SEL4 = np.ascontiguousarray(_SEL4.reshape(4, 4 * DH))

_NC_CACHE = {}
_TRACE = False
_LAST_EXEC_NS = None


def _get_nc():
    if "nc" not in _NC_CACHE:
        _NC_CACHE["nc"] = build_nc()
    return _NC_CACHE["nc"]


def make_in_maps(inputs_q, inputs_kv, attention_mask, Wq, bq, Wk, bk, Wv, bv, Wp):
    """Per-core input dicts (shared by kernel() and test.py's bench)."""
    in_maps = []
    for c in range(NCORES):
        bidx, g = divmod(c, HG)
        cs = slice(g * CSL, (g + 1) * CSL)
        in_maps.append(
            {
                "xqT": np.ascontiguousarray(inputs_q[bidx].T.astype(np.float16)),
                "xkvT": np.ascontiguousarray(inputs_kv[bidx].T.astype(np.float16)),
                "maskT": np.ascontiguousarray(
                    attention_mask[bidx, 0].T.astype(np.float16)
                ),
                "wq": np.ascontiguousarray(Wq[:, cs].astype(np.float16)),
                "wk": np.ascontiguousarray(Wk[:, cs].astype(np.float16)),
                "wv": np.ascontiguousarray(Wv[:, cs].astype(np.float16)),
                "wp": np.ascontiguousarray(Wp[cs, :].astype(np.float16)),
                "bq2": np.ascontiguousarray(
                    bq[cs].astype(np.float32).reshape(2, 128).T
                ),
                "bk2": np.ascontiguousarray(
                    bk[cs].astype(np.float32).reshape(2, 128).T
                ),
                "bv1": np.ascontiguousarray(bv[cs].reshape(1, CSL).astype(np.float16)),
                "sel4in": _SEL4,
            }
        )
    return in_maps


def kernel(
    inputs_kv, inputs_q, attention_mask, Wq, bq, Wk, bk, Wv, bv, Wp, bp, **_unused
):
    inputs_kv = np.asarray(inputs_kv, dtype=np.float32)
    inputs_q = np.asarray(inputs_q, dtype=np.float32)
    attention_mask = np.asarray(attention_mask)
    bp = np.asarray(bp, dtype=np.float32)

    in_maps = make_in_maps(
        inputs_q, inputs_kv, attention_mask,
        np.asarray(Wq, dtype=np.float32), np.asarray(bq, dtype=np.float32),
        np.asarray(Wk, dtype=np.float32), np.asarray(bk, dtype=np.float32),
        np.asarray(Wv, dtype=np.float32), np.asarray(bv, dtype=np.float32),
        np.asarray(Wp, dtype=np.float32),
    )

    nc = _get_nc()
    res = run_bass_kernel_spmd(
        nc, in_maps, core_ids=list(range(NCORES)), trace=_TRACE
    )
    global _LAST_EXEC_NS
    _LAST_EXEC_NS = res.exec_time_ns

    out = np.zeros((B, N, D), dtype=np.float32)
    for c in range(NCORES):
        bidx = c // HG
        out[bidx] += res.results[c]["outp"].astype(np.float32)
    out += bp
    return out


# revision 9
# speedup vs baseline: 1745.3256x; 1.0679x over previous
"""Trainium2 Bass kernel for nn_MultiHeadAttention (B=2, N=M=2048, D=1024, H=16).

Sharding: 8 cores = 2 batches x 4 head-groups (4 heads per core, tensor-parallel
over the head dim of Wq/Wk/Wv/Wp).  Each core computes a partial output
projection [N, D]; the host sums the 4 partials per batch and adds bp.

v3 dataflow (ACT-saturating pipeline; ACT's exp of the 4 x 2048 x 2048 score
matrix at 1 elem/lane/cycle is the hard floor, ~128us/core):
  - attention runs over 4 n-strips of 512.  Per (strip, g, m-chunk): two
    64-contraction score matmuls (heads 2g / 2g+1, PE row-tiles at partition
    bases 0/64 run them concurrently) into a double-buffered [128, 2, 512]
    PSUM pair; ONE ACT exp op over the full [128, 1024] with the 1/sqrt(dh)
    scale folded in; ONE DVE f16 mask multiply against a host-duplicated
    [m, 2, n] mask strip; two AV matmuls accumulating [65, 512] per head
    (ones-column = softmax denominator).
  - everything else rides in the PE/DVE slack under ACT via a thunk queue
    drained one item per m-iteration: the K/V/Q projections (prelude work
    interleaves into strip 0 so the first exp fires ~10us in; an up-front
    dummy exp preloads the ACT table during the DMA ramp), the Q^T
    projection for strip s+1, and strip s's tail (denominator gather,
    PE-transpose -> f16 reciprocal -> broadcast matmul -> in-place
    normalize, then the output projection as 4 x K=64 matmuls per [128,512]
    tile).  The queue carries across reps, so rep r's last tail overlaps
    rep r+1's prelude.
"""

import os
import numpy as np
from contextlib import ExitStack

import concourse.bass as bass
import concourse.tile as tile
from concourse import mybir
from concourse.bass_utils import run_bass_kernel_spmd
from concourse.vector_clock import ScopedClock
from concourse.masks import make_identity

B, N, M, D = 2, 2048, 2048, 1024
H = 16
DH = D // H  # 64
SCALE = DH ** -0.5
NCORES = 8
HG = 4            # heads per core
CSL = HG * DH     # 256 columns of Wq/Wk/Wv per core
F32 = mybir.dt.float32
F16 = mybir.dt.float16

NSTRIP = 512          # attention n-strip width
NS = N // NSTRIP      # 4 strips
MT = M // 128         # 16 m-chunks

_ABLATE = os.environ.get("ABLATE", "")  # comma-list: nomask, noexp

# ---------------------------------------------------------------------------
# walrus in this container rejects >1 sem wait per instruction; spread the
# extras across preceding same-engine NOPs (queues execute in order, so this
# is semantically identical).
_MAX_WAITS = 1


def _patched_drain_and_barrier(self, tick_clock, wait_clock):
    drain_inst = self.nc.sync.drain()
    wait_clock.add_sem_waits(
        drain_inst.ins, ScopedClock({None: tick_clock.global_clock})
    )
    si = drain_inst.ins.sync_info
    waits = list(si.on_wait or []) if si else []
    if len(waits) > _MAX_WAITS:
        si.on_wait = waits[:_MAX_WAITS]
        for i in range(_MAX_WAITS, len(waits), _MAX_WAITS):
            extra = self.nc.sync.drain()
            extra.ins.sync_info = mybir.SyncInfo(
                on_wait=waits[i : i + _MAX_WAITS], on_update=[]
            )
    self.nc.all_engine_barrier()
    assert self.sems is not None
    popped = self.nc._tile_sem_poison_stack.pop()
    assert popped is self._sem_poison
    self.nc.clear_and_free_semaphores(list(self.sems.allocated().values()))
    self.nc.all_engine_barrier()


tile.TileContext._drain_and_barrier = _patched_drain_and_barrier
# ---------------------------------------------------------------------------

Exp = mybir.ActivationFunctionType.Exp
Identity = mybir.ActivationFunctionType.Identity


def _split_waits(nc):
    n_split = 0
    for bb in nc.main_func.blocks:
        new_list = []
        for ins in bb.instructions:
            si = ins.sync_info
            if si is not None and si.on_wait and len(si.on_wait) > 1:
                waits = list(si.on_wait)
                for j, w in enumerate(waits[:-1]):
                    nop = mybir.InstNoOp(
                        name=f"{ins.name}-sw{j}",
                        engine=ins.engine,
                        sync_info=mybir.SyncInfo(on_wait=[w], on_update=[]),
                    )
                    new_list.append(nop)
                    n_split += 1
                si.on_wait = [waits[-1]]
            new_list.append(ins)
        bb.instructions = new_list
    return n_split


def build_nc(reps: int = 1) -> bass.Bass:
    nc = bass.Bass()

    xqT = nc.dram_tensor("xqT", [D, N], F16, kind="ExternalInput")
    xkvT = nc.dram_tensor("xkvT", [D, M], F16, kind="ExternalInput")
    maskdT = nc.dram_tensor("maskdT", [M, 2, N], F16, kind="ExternalInput")
    wq = nc.dram_tensor("wq", [D, CSL], F16, kind="ExternalInput")
    wk = nc.dram_tensor("wk", [D, CSL], F16, kind="ExternalInput")
    wv = nc.dram_tensor("wv", [D, CSL], F16, kind="ExternalInput")
    wp = nc.dram_tensor("wp", [CSL, D], F16, kind="ExternalInput")
    bq2 = nc.dram_tensor("bq2", [128, 2], F32, kind="ExternalInput")
    bk2 = nc.dram_tensor("bk2", [128, 2], F32, kind="ExternalInput")
    bv1 = nc.dram_tensor("bv1", [1, CSL], F16, kind="ExternalInput")
    sel4in = nc.dram_tensor("sel4in", [4, HG * DH], F16, kind="ExternalInput")
    outp = nc.dram_tensor("outp", [N, D], F16, kind="ExternalOutput")

    with ExitStack() as ctx:
        tc = ctx.enter_context(tile.TileContext(nc))

        consts = ctx.enter_context(tc.tile_pool(name="consts", bufs=1))
        ident = consts.tile([128, 128], F16)
        make_identity(nc, ident)
        ones_row = consts.tile([1, 128], F16)
        nc.vector.memset(ones_row, 1.0)
        sel4 = consts.tile([4, 4, DH], F16)
        nc.sync.dma_start(out=sel4, in_=sel4in[:, :])
        bq_sb = consts.tile([128, 2], F32)
        nc.sync.dma_start(out=bq_sb, in_=bq2[:, :])
        bk_sb = consts.tile([128, 2], F32)
        nc.sync.dma_start(out=bk_sb, in_=bk2[:, :])
        bv_sb = consts.tile([1, CSL], F16)
        nc.sync.dma_start(out=bv_sb, in_=bv1[:, :])
        # wp per head at partitions 0-63: [64, h, slice, 512]
        wp_sb = consts.tile([DH, HG, 2, 512], F16)
        for h in range(HG):
            nc.sync.dma_start(
                out=wp_sb[:, h, :, :], in_=wp[h * DH : (h + 1) * DH, :]
            )
        # preload the exp table set while the first DMAs run
        warm = consts.tile([1, 2], F32)
        nc.vector.memset(warm, 0.0)
        nc.scalar.activation(warm, warm, Exp, scale=1.0)

        persist = ctx.enter_context(tc.tile_pool(name="persist", bufs=1))
        KT = [persist.tile([128, M], F16, tag=f"kt{g}", name=f"kt{g}") for g in range(2)]
        QT = [persist.tile([128, N], F16, tag=f"qt{g}", name=f"qt{g}") for g in range(2)]
        V = persist.tile([128, MT, HG, DH + 1], F16, tag="v")
        wq_sb = persist.tile([128, 8, CSL], F16, tag="wq")
        for d in range(8):
            nc.sync.dma_start(out=wq_sb[:, d, :], in_=wq[d * 128 : (d + 1) * 128, :])

        maskp = ctx.enter_context(tc.tile_pool(name="maskp", bufs=2))
        xqp = ctx.enter_context(tc.tile_pool(name="xqp", bufs=2))
        xkvp = ctx.enter_context(tc.tile_pool(name="xkvp", bufs=2))
        wkvp = ctx.enter_context(tc.tile_pool(name="wkvp", bufs=1))
        etap = ctx.enter_context(tc.tile_pool(name="etap", bufs=4))
        otsp = ctx.enter_context(tc.tile_pool(name="otsp", bufs=2))
        obp = ctx.enter_context(tc.tile_pool(name="obp", bufs=2))
        nrmp = ctx.enter_context(tc.tile_pool(name="nrmp", bufs=2))

        stp = ctx.enter_context(tc.tile_pool(name="stp", bufs=2, space="PSUM"))
        otp = ctx.enter_context(tc.tile_pool(name="otp", bufs=1, space="PSUM"))
        tailp = ctx.enter_context(tc.tile_pool(name="tailp", bufs=2, space="PSUM"))

        pending = []  # emission thunks drained one per m-iteration

        def dma_xq_strip(s):
            xq_sb = xqp.tile([128, 8, NSTRIP], F16, tag="xq", name=f"xq{s}")
            for d in range(8):
                nc.sync.dma_start(
                    out=xq_sb[:, d, :],
                    in_=xqT[d * 128 : (d + 1) * 128, s * NSTRIP : (s + 1) * NSTRIP],
                )
            return xq_sb

        def dma_mask_strip(s):
            mkd = maskp.tile([128, MT, 2, 512], F16, tag="mk", name=f"mk{s}")
            for m in range(MT):
                nc.sync.dma_start(
                    out=mkd[:, m, :, :],
                    in_=maskdT[
                        m * 128 : (m + 1) * 128, :, s * NSTRIP : (s + 1) * NSTRIP
                    ],
                )
            return mkd

        def project_q(xq_sb, s, on_act=False):
            for g in range(2):
                ps = tailp.tile([128, NSTRIP], F32, tag="t", name=f"qps{s}{g}")
                for d in range(8):
                    nc.tensor.matmul(
                        ps,
                        wq_sb[:, d, g * 128 : (g + 1) * 128],
                        xq_sb[:, d, :],
                        start=(d == 0),
                        stop=(d == 7),
                    )
                dst = QT[g][:, s * NSTRIP : (s + 1) * NSTRIP]
                if on_act:
                    nc.scalar.activation(dst, ps, Identity, bias=bq_sb[:, g : g + 1])
                else:
                    nc.vector.tensor_scalar_add(dst, ps, bq_sb[:, g : g + 1])

        def project_k_tile(wk_sb, xkv_sb, g, mh, ms, on_act=False):
            ps = tailp.tile([128, 512], F32, tag="t", name="kps")
            for d in range(8):
                nc.tensor.matmul(
                    ps,
                    wk_sb[:, d, g * 128 : (g + 1) * 128],
                    xkv_sb[:, d, ms * 512 : (ms + 1) * 512],
                    start=(d == 0),
                    stop=(d == 7),
                )
            dst = KT[g][:, mh * 1024 + ms * 512 : mh * 1024 + (ms + 1) * 512]
            if on_act:
                nc.scalar.activation(dst, ps, Identity, bias=bk_sb[:, g : g + 1])
            else:
                nc.vector.tensor_scalar_add(dst, ps, bk_sb[:, g : g + 1])

        def project_v_pair(wv_sb, xkv_sb, mh, mt0):
            for mt in (mt0, mt0 + 1):
                vp = tailp.tile([128, HG, DH], F32, tag="t", name="vp")
                for d in range(8):
                    nc.tensor.matmul(
                        vp,
                        xkv_sb[:, d, mt * 128 : (mt + 1) * 128],
                        wv_sb[:, d, :],
                        start=(d == 0),
                        stop=False,
                    )
                nc.tensor.matmul(vp, ones_row, bv_sb, start=False, stop=True)
                nc.vector.tensor_copy(V[:, mh * 8 + mt, :, 0:DH], vp)

        def emit_tail(s, ots):
            """Normalize + output projection for strip s, as thunks."""

            def norm_chain(s=s, ots=ots):
                rowsums = nrmp.tile([4, NSTRIP], F16, tag="rs", name="rs")
                nc.sync.dma_start(out=rowsums, in_=ots[DH : DH + 1, :, :])
                rsT = tailp.tile([128, 16], F16, tag="t", name="rsT")
                for b in range(4):
                    nc.tensor.transpose(
                        rsT[:, 4 * b : 4 * b + 4],
                        rowsums[:, b * 128 : (b + 1) * 128],
                        ident[0:4, 0:4],
                    )
                recipT = nrmp.tile([128, 16], F16, tag="rcp", name="recipT")
                with nc.allow_low_precision(
                    reason="softmax denominators are O(1e3); f16 recip gives "
                    "~5e-4 rel err, well inside the output tolerance"
                ):
                    nc.vector.reciprocal(recipT, rsT)
                rrps = tailp.tile([4, NSTRIP], F16, tag="t", name="rrps")
                for b in range(4):
                    nc.tensor.transpose(
                        rrps[:, b * 128 : (b + 1) * 128],
                        recipT[:, 4 * b : 4 * b + 4],
                        ident,
                    )
                recrows = nrmp.tile([4, NSTRIP], F16, tag="rr", name="recrows")
                nc.vector.tensor_copy(recrows, rrps)
                emit_tail.recrows = recrows

            def norm_heads(h0, s=s, ots=ots):
                recrows = emit_tail.recrows
                for h in (h0, h0 + 1):
                    rps = tailp.tile([DH, NSTRIP], F32, tag="t", name="rps")
                    nc.tensor.matmul(rps, sel4[:, h, :], recrows)
                    nc.vector.tensor_mul(ots[0:DH, h, :], ots[0:DH, h, :], rps)

            def out_tile(t, s=s, ots=ots):
                ob = obp.tile([128, 2, 512], F16, tag="ob", name="ob")
                for sl in range(2):
                    po = tailp.tile([128, 512], F32, tag="t", name="po")
                    for h in range(HG):
                        nc.tensor.matmul(
                            po,
                            ots[0:DH, h, t * 128 : (t + 1) * 128],
                            wp_sb[:, h, sl, :],
                            start=(h == 0),
                            stop=(h == HG - 1),
                        )
                    if (t * 2 + sl) % 4 == 3:
                        nc.scalar.copy(ob[:, sl, :], po)
                    else:
                        nc.vector.tensor_copy(ob[:, sl, :], po)
                row = s * NSTRIP + t * 128
                nc.sync.dma_start(out=outp[row : row + 128, :], in_=ob)

            pending.append(norm_chain)
            pending.append(lambda: norm_heads(0))
            pending.append(lambda: norm_heads(2))
            for t in range(NSTRIP // 128):
                pending.append(lambda t=t: out_tile(t))

        for _rep in range(reps):
            # ---- prelude: DMAs + enough projections to start strip 0 ----
            wk_sb = wkvp.tile([128, 8, CSL], F16, tag="wk", name="wk_sb")
            wv_sb = wkvp.tile([128, 8, CSL], F16, tag="wv", name="wv_sb")
            for d in range(8):
                nc.sync.dma_start(out=wk_sb[:, d, :], in_=wk[d * 128 : (d + 1) * 128, :])
                nc.sync.dma_start(out=wv_sb[:, d, :], in_=wv[d * 128 : (d + 1) * 128, :])
            xq0 = dma_xq_strip(0)
            xkv = []
            for mh in range(2):
                xkv_sb = xkvp.tile([128, 8, 1024], F16, tag="xkv", name=f"xkv{mh}")
                for d in range(8):
                    nc.sync.dma_start(
                        out=xkv_sb[:, d, :],
                        in_=xkvT[d * 128 : (d + 1) * 128, mh * 1024 : (mh + 1) * 1024],
                    )
                xkv.append(xkv_sb)
            mk_strip = dma_mask_strip(0)

            nc.vector.memset(V[:, :, :, DH : DH + 1], 1.0)
            project_q(xq0, 0, on_act=True)
            project_k_tile(wk_sb, xkv[0], 0, 0, 0, on_act=True)
            project_v_pair(wv_sb, xkv[0], 0, 0)
            project_v_pair(wv_sb, xkv[0], 0, 2)

            # PREPEND: carried tail thunks from the previous rep must not
            # push these past the m-iterations whose scores depend on them
            # (PE executes in queue order -> that would deadlock).
            def _kt(xh, g, mh, ms, act=False, wk_sb=wk_sb, xkv=xkv):
                return lambda: project_k_tile(wk_sb, xkv[xh], g, mh, ms, on_act=act)

            def _vp(xh, mh, mt0, wv_sb=wv_sb, xkv=xkv):
                return lambda: project_v_pair(wv_sb, xkv[xh], mh, mt0)

            pending[:0] = [
                _kt(0, 0, 0, 1, act=True),
                _vp(0, 0, 4),
                _vp(0, 0, 6),
                _kt(0, 1, 0, 0),
                _kt(0, 1, 0, 1),
                _kt(1, 0, 1, 0),
                _vp(1, 1, 0),
                _vp(1, 1, 2),
                _kt(1, 0, 1, 1),
                _vp(1, 1, 4),
                _vp(1, 1, 6),
                _kt(1, 1, 1, 0),
                _kt(1, 1, 1, 1),
            ]

            # ---------------- attention over n-strips --------------------
            for s in range(NS):
                mkd = mk_strip
                if s + 1 < NS:
                    mk_strip = dma_mask_strip(s + 1)

                ots = otsp.tile([DH + 1, HG, NSTRIP], F16, tag="ots", name=f"ots{s}")
                for g in range(2):
                    otg = otp.tile([DH + 1, 2, NSTRIP], F32, tag="ot", name=f"ot{s}{g}")
                    for m in range(MT):
                        s2 = stp.tile([128, 2, NSTRIP], F32, tag="s2", name="s2")
                        for i in range(2):
                            nc.tensor.matmul(
                                s2[:, i, :],
                                KT[g][i * 64 : (i + 1) * 64, m * 128 : (m + 1) * 128],
                                QT[g][
                                    i * 64 : (i + 1) * 64,
                                    s * NSTRIP : (s + 1) * NSTRIP,
                                ],
                            )
                        eta = etap.tile([128, 2, NSTRIP], F16, tag="eta", name="eta")
                        if "noexp" in _ABLATE:
                            nc.vector.tensor_copy(eta, s2)
                        else:
                            nc.scalar.activation(eta, s2, Exp, scale=SCALE)
                        if "nomask" not in _ABLATE:
                            nc.vector.tensor_mul(eta, eta, mkd[:, m, :, :])
                        for i in range(2):
                            nc.tensor.matmul(
                                otg[:, i, :],
                                V[:, m, 2 * g + i, :],
                                eta[:, i, :],
                                start=(m == 0),
                                stop=(m == MT - 1),
                            )
                        if pending:
                            pending.pop(0)()
                    for i in range(2):
                        nc.vector.tensor_copy(
                            ots[:, 2 * g + i, :], otg[0 : DH + 1, i, :]
                        )
                    if g == 0 and s + 1 < NS:
                        xq_nxt = dma_xq_strip(s + 1)
                        pending.append(lambda xq=xq_nxt, ss=s + 1: project_q(xq, ss))

                emit_tail(s, ots)

        while pending:
            pending.pop(0)()

    _split_waits(nc)
    return nc


_SEL4 = np.zeros((4, 4, DH), dtype=np.float16)
for _h in range(4):
    _SEL4[_h, _h, :] = 1.0
_SEL4 = np.ascontiguousarray(_SEL4.reshape(4, 4 * DH))

_NC_CACHE = {}
_TRACE = False
_LAST_EXEC_NS = None


def _get_nc():
    if "nc" not in _NC_CACHE:
        _NC_CACHE["nc"] = build_nc()
    return _NC_CACHE["nc"]


def make_in_maps(inputs_q, inputs_kv, attention_mask, Wq, bq, Wk, bk, Wv, bv, Wp):
    """Per-core input dicts (shared by kernel() and test.py's bench)."""
    in_maps = []
    for c in range(NCORES):
        bidx, g = divmod(c, HG)
        cs = slice(g * CSL, (g + 1) * CSL)
        maskT = attention_mask[bidx, 0].T.astype(np.float16)  # [M, N]
        maskd = np.ascontiguousarray(
            np.repeat(maskT[:, None, :], 2, axis=1)
        )  # [M, 2, N]
        in_maps.append(
            {
                "xqT": np.ascontiguousarray(inputs_q[bidx].T.astype(np.float16)),
                "xkvT": np.ascontiguousarray(inputs_kv[bidx].T.astype(np.float16)),
                "maskdT": maskd,
                "wq": np.ascontiguousarray(Wq[:, cs].astype(np.float16)),
                "wk": np.ascontiguousarray(Wk[:, cs].astype(np.float16)),
                "wv": np.ascontiguousarray(Wv[:, cs].astype(np.float16)),
                "wp": np.ascontiguousarray(Wp[cs, :].astype(np.float16)),
                "bq2": np.ascontiguousarray(
                    bq[cs].astype(np.float32).reshape(2, 128).T
                ),
                "bk2": np.ascontiguousarray(
                    bk[cs].astype(np.float32).reshape(2, 128).T
                ),
                "bv1": np.ascontiguousarray(bv[cs].reshape(1, CSL).astype(np.float16)),
                "sel4in": _SEL4,
            }
        )
    return in_maps


def kernel(
    inputs_kv, inputs_q, attention_mask, Wq, bq, Wk, bk, Wv, bv, Wp, bp, **_unused
):
    inputs_kv = np.asarray(inputs_kv, dtype=np.float32)
    inputs_q = np.asarray(inputs_q, dtype=np.float32)
    attention_mask = np.asarray(attention_mask)
    bp = np.asarray(bp, dtype=np.float32)

    in_maps = make_in_maps(
        inputs_q, inputs_kv, attention_mask,
        np.asarray(Wq, dtype=np.float32), np.asarray(bq, dtype=np.float32),
        np.asarray(Wk, dtype=np.float32), np.asarray(bk, dtype=np.float32),
        np.asarray(Wv, dtype=np.float32), np.asarray(bv, dtype=np.float32),
        np.asarray(Wp, dtype=np.float32),
    )

    nc = _get_nc()
    res = run_bass_kernel_spmd(
        nc, in_maps, core_ids=list(range(NCORES)), trace=_TRACE
    )
    global _LAST_EXEC_NS
    _LAST_EXEC_NS = res.exec_time_ns

    out = np.zeros((B, N, D), dtype=np.float32)
    for c in range(NCORES):
        bidx = c // HG
        out[bidx] += res.results[c]["outp"].astype(np.float32)
    out += bp
    return out
